# revision 1
# baseline (speedup 1.0000x reference)
"""Bass/Trainium2 kernel for fused bilinear attention + softmax.

reference computation:
    pa = a @ Wa + ba                      (B, La, D)
    pb = b @ Wb + bb                      (B, Lb, D)
    scores = einsum('bid,bjd->bij', pa * w, pb) + wbias
    out = softmax(scores.reshape(B, La*Lb)).reshape(B, La, Lb)

Device strategy (8 NeuronCores, data-parallel over batch, 8 batches/core):
    Weight-only host folding:  M = (Wa*w) @ Wb.T,  u = (Wa*w)@bb,  v = (Wb*w)@ba
      scores[b,i,j] = a_i M b_j^T + (a_i.u) + (b_j.v) + const
    const (+wbias) is dropped: softmax over the flattened grid is shift-invariant.
    Host pre-transposes a,b to feature-major bf16 (aT, bT), so no on-device
    transposes are needed.  Per pair of batches (rhs free dim 512):
      TT   = M @ bT + u       64 bf16 matmuls (N=512); DVE eviction adds u
      bu   = v . bT           DVE multiply-accumulate + one PE reduce matmul
      S    = aT^T @ TT + 1(x)bu  36 bf16 matmuls (N=256), bu via K=1 matmul
      softmax: fused ACT exp+rowsum (accum_out) -> GpSimd partition_all_reduce
               -> DVE reciprocal -> DVE scale -> DMA out
    PE warm-up matmuls run during the initial DMAs (HAM clock-gate release).
"""

import numpy as np
import ml_dtypes

import concourse.bass as bass
import concourse.bacc as bacc
import concourse.mybir as mybir
import concourse.tile as tile
from concourse.bass_utils import run_bass_kernel_spmd

BF16 = ml_dtypes.bfloat16

N_CORES = 8
B, L, K = 64, 256, 1024          # batch, seq len (La=Lb), feature dim (IN_A=IN_B)
BPC = B // N_CORES               # batches per core
G = BPC // 2                     # batch-pair groups per core
KC = K // 128                    # feature chunks of 128
F32 = mybir.dt.float32
DBF = mybir.dt.bfloat16
Act = mybir.ActivationFunctionType


def _build_program():
    # Bacc (not raw Bass): its compile() legalizes multi-wait instructions
    # (TRN2 allows at most one sync wait per instruction).
    nc = bacc.Bacc("TRN2", debug=False, target_bir_lowering=False)

    at = nc.dram_tensor("at", [G, K, 2 * L], DBF, kind="ExternalInput")
    bt = nc.dram_tensor("bt", [G, K, 2 * L], DBF, kind="ExternalInput")
    # M^T in m-major blocks: mt[m, p, l_chunk, ki] = M.T[l_chunk*128+p, m*128+ki]
    mt = nc.dram_tensor("mt", [KC, 128, KC, 128], DBF, kind="ExternalInput")
    u = nc.dram_tensor("u", [K], F32, kind="ExternalInput")
    v = nc.dram_tensor("v", [K], F32, kind="ExternalInput")
    probs = nc.dram_tensor("probs", [BPC, L, L], F32, kind="ExternalOutput")

    with tile.TileContext(nc) as tc:
        with (
            tc.tile_pool(name="consts", bufs=1) as consts,
            tc.tile_pool(name="inp", bufs=2) as in_pool,
            tc.tile_pool(name="tt", bufs=10) as tt_pool,
            tc.tile_pool(name="sm", bufs=4) as sm_pool,
            tc.tile_pool(name="small", bufs=4) as small,
            tc.tile_pool(name="ps_tt", bufs=5, space="PSUM") as ps_tt,
            tc.tile_pool(name="ps_sc", bufs=2, space="PSUM") as ps_sc,
            tc.tile_pool(name="ps_bu", bufs=1, space="PSUM") as ps_bu,
        ):
            # ---- constants (chunked DMAs so PE can start on chunk 0 early) ----
            u_sb = consts.tile([128, KC], F32)              # u[c*128+p] at [p, c]
            nc.sync.dma_start(out=u_sb, in_=u[:].rearrange("(c p) -> p c", p=128))
            v_sb = consts.tile([128, KC], F32)
            nc.sync.dma_start(out=v_sb, in_=v[:].rearrange("(c p) -> p c", p=128))
            mt_sb = consts.tile([128, KC, KC, 128], DBF)    # [l_in, m, l_chunk, ki]
            ones_col_f32 = consts.tile([128, 1], F32)
            nc.vector.memset(ones_col_f32, 1.0)
            ones_row_f32 = consts.tile([1, 128], F32)
            nc.vector.memset(ones_row_f32, 1.0)
            ones_row_bf = consts.tile([1, 128], DBF)
            nc.vector.memset(ones_row_bf, 1.0)
            ones_col_bf = consts.tile([128, 1], DBF)
            nc.vector.memset(ones_col_bf, 1.0)

            from concourse import library_config
            nc.gpsimd.load_library(library_config.attnmlp)

            # PE warm-up: dummy matmuls while the first DMAs land, so the HAM
            # clock gate is already released when real matmuls start.
            warm_sb = consts.tile([128, 2 * L], DBF)
            nc.vector.memset(warm_sb, 0.0)
            warm_ps = ps_bu.tile([128, 2 * L], F32, tag="bu_ps")
            for i in range(5):
                nc.tensor.matmul(
                    warm_ps, warm_sb[:, 0:128], warm_sb,
                    start=(i == 0), stop=(i == 4),
                )

            for g in range(G):
                bt_sb = in_pool.tile([128, KC, 2 * L], DBF, tag="bt")
                for l in range(KC):
                    if g == 0:
                        # interleave M^T m-blocks with the first group's bT so
                        # the TT accumulation can start as blocks arrive
                        nc.sync.dma_start(out=mt_sb[:, l], in_=mt[l])
                    nc.sync.dma_start(
                        out=bt_sb[:, l, :], in_=bt[g, l * 128 : (l + 1) * 128, :]
                    )
                at_sb = in_pool.tile([128, KC, 2 * L], DBF, tag="at")
                for l in range(KC):
                    nc.sync.dma_start(
                        out=at_sb[:, l, :], in_=at[g, l * 128 : (l + 1) * 128, :]
                    )

                # Phase 1: all 8 TT chunks (kept in SBUF; tt_pool holds them all)
                tt_chunks = []
                for m in range(KC):
                    # TT chunk m: rows k in [128m, 128m+128), all 512 cols
                    tt_ps = ps_tt.tile([128, 2 * L], F32, tag="tt_ps")
                    for l in range(KC):
                        nc.tensor.matmul(
                            tt_ps, mt_sb[:, m, l, :], bt_sb[:, l, :],
                            start=(l == 0), stop=(l == KC - 1),
                        )
                    tt_sb = tt_pool.tile([128, 2 * L], DBF, tag="tt")
                    # TT' = TT + u[chunk m] (folds the a.u rank-1 term); DVE
                    # (not ACT) so the scalar engine never swaps LUT tables.
                    nc.vector.tensor_scalar_add(tt_sb, tt_ps, u_sb[:, m : m + 1])
                    tt_chunks.append(tt_sb)
                    # one bu MAC step per chunk, AFTER the eviction in DVE
                    # order: evictions gate PSUM recycling, bu is slack work.
                    # bu[j] = sum_l v[l] * bT[l, j]
                    if m == 0:
                        bu_acc = tt_pool.tile([128, 2 * L], DBF, tag="buacc")
                        nc.vector.tensor_scalar_mul(
                            bu_acc, bt_sb[:, 0, :], v_sb[:, 0:1]
                        )
                    else:
                        nc.vector.scalar_tensor_tensor(
                            bu_acc, bt_sb[:, m, :], v_sb[:, m : m + 1], bu_acc,
                            op0=mybir.AluOpType.mult, op1=mybir.AluOpType.add,
                        )

                # reduce bu over partitions (one PE matmul) and stage as bf16
                bu_ps = ps_bu.tile([1, 2 * L], F32, tag="bu_ps")
                nc.tensor.matmul(
                    bu_ps, ones_col_bf, bu_acc, start=True, stop=True
                )
                bu_sb = small.tile([1, 2 * L], DBF, tag="bu")
                nc.vector.tensor_copy(out=bu_sb, in_=bu_ps)

                # Phase 2: scores per batch in ONE psum bank (sequential h
                # accumulation groups), then a single fused exp+rowsum.
                for q in range(2):
                    sc_ps = ps_sc.tile([128, 2 * L], F32, tag="sc")
                    for h in range(2):
                        for m in range(KC):
                            nc.tensor.matmul(
                                sc_ps[:, h * L : (h + 1) * L],
                                at_sb[:, m, q * L + h * 128 : q * L + h * 128 + 128],
                                tt_chunks[m][:, q * L : (q + 1) * L],
                                start=(m == 0), stop=False,
                            )
                        # inject bu (K=1 accumulate): S[i, j] += 1 * bu[j]
                        nc.tensor.matmul(
                            sc_ps[:, h * L : (h + 1) * L],
                            ones_row_bf, bu_sb[:, q * L : (q + 1) * L],
                            start=False, stop=True,
                        )

                    # ---- softmax over the whole (256, 256) grid per batch ----
                    exp_sb = sm_pool.tile([128, 2 * L], F32, tag="exp")
                    colsum = small.tile([128, 1], F32, tag="cs")
                    nc.scalar.activation(
                        exp_sb, sc_ps, Act.Exp, accum_out=colsum
                    )
                    # total over partitions, broadcast to all (GpSimd), recip
                    tot_col = small.tile([128, 1], F32, tag="totc")
                    nc.gpsimd.partition_all_reduce(
                        tot_col, colsum, channels=128,
                        reduce_op=bass.bass_isa.ReduceOp.add,
                    )
                    rcp_col = small.tile([128, 1], F32, tag="rcpc")
                    nc.vector.reciprocal(rcp_col, tot_col)
                    probs_sb = sm_pool.tile([128, 2 * L], F32, tag="probs")
                    for h in range(2):
                        # split by half so the first DMA overlaps the second mul
                        nc.vector.tensor_scalar_mul(
                            probs_sb[:, h * L : (h + 1) * L],
                            exp_sb[:, h * L : (h + 1) * L],
                            rcp_col,
                        )
                        nc.sync.dma_start(
                            out=probs[2 * g + q][h * 128 : (h + 1) * 128, :],
                            in_=probs_sb[:, h * L : (h + 1) * L],
                        )
    return nc


def _prep_host(a, b, Wa, ba, Wb, bb, w, wbias):
    """Weight folding (f64) + per-core feature-major bf16 shards."""
    Wa64 = Wa.astype(np.float64)
    Wb64 = Wb.astype(np.float64)
    w64 = w.astype(np.float64)
    M = (Wa64 * w64[None, :]) @ Wb64.T                  # (K, K)
    u_np = ((Wa64 * w64[None, :]) @ bb.astype(np.float64)).astype(np.float32)
    v_np = ((Wb64 * w64[None, :]) @ ba.astype(np.float64)).astype(np.float32)
    # m-major blocked M^T: mt[m, p, c, ki] = M.T[c*128+p, m*128+ki]
    mt_np = np.ascontiguousarray(
        M.T.astype(np.float32)
        .reshape(KC, 128, KC, 128)
        .transpose(2, 1, 0, 3)
    ).astype(BF16)

    def shard(x):
        # (BPC, L, K) -> (G, K, 2L) feature-major bf16, batch pairs side by side
        xt = x.transpose(0, 2, 1)                        # (BPC, K, L)
        xt = xt.reshape(G, 2, K, L).transpose(0, 2, 1, 3).reshape(G, K, 2 * L)
        return np.ascontiguousarray(xt).astype(BF16)

    in_maps = []
    for c in range(N_CORES):
        sl = slice(c * BPC, (c + 1) * BPC)
        in_maps.append(
            {
                "at": shard(a[sl]),
                "bt": shard(b[sl]),
                "mt": mt_np,
                "u": u_np,
                "v": v_np,
            }
        )
    return in_maps


def _run(inputs, trace=False):
    nc = _build_program()
    nc.compile()
    in_maps = _prep_host(**inputs)
    res = run_bass_kernel_spmd(
        nc, in_maps, core_ids=list(range(N_CORES)), trace=trace
    )
    out = np.concatenate([res.results[c]["probs"] for c in range(N_CORES)], axis=0)
    return out.astype(np.float32), res


def kernel(**inputs) -> np.ndarray:
    out, _ = _run(inputs, trace=False)
    return out



# revision 2
# speedup vs baseline: 1.1821x; 1.1821x over previous
"""Bass/Trainium2 kernel for fused bilinear attention + softmax.

reference computation:
    pa = a @ Wa + ba                      (B, La, D)
    pb = b @ Wb + bb                      (B, Lb, D)
    scores = einsum('bid,bjd->bij', pa * w, pb) + wbias
    out = softmax(scores.reshape(B, La*Lb)).reshape(B, La, Lb)

Device strategy (8 NeuronCores, data-parallel over batch, 8 batches/core):
    Weight-only host folding:  M = (Wa*w) @ Wb.T,  u = (Wa*w)@bb,  v = (Wb*w)@ba
      scores[b,i,j] = a_i M b_j^T + (a_i.u) + (b_j.v) + const
    const (+wbias) dropped: softmax over the flattened grid is shift-invariant.
    bu[b,j] = v . b_j is a rank-1 term computed on host (like u/v folding).

    Everything device-side carries a power-of-2 scale sM on M (so the fp8
    chunks use the e4m3 range); exp() unscales via its scale operand.

    Per pair of batches (rhs free dim 512):
      TT   = (sM*M) @ bT + sM*u   mixed-precision contraction:
             first N8 feature chunks as fp8e4m3 DoubleRow pair-matmuls
             (2 chunks per instruction, 2x PE throughput), the rest bf16.
             DVE eviction to bf16 adds sM*u.
      S    = aT^T @ TT + 1(x)(sM*bu)  bf16 matmuls (N=256) + K=1 inject
      softmax: ACT exp(S/sM) with accum_out rowsum -> GpSimd
               partition_all_reduce -> DVE reciprocal -> DVE scale -> DMA out
    PE warm-up matmuls run during the initial DMAs (HAM clock-gate release).
"""

import numpy as np
import ml_dtypes

import concourse.bass as bass
import concourse.bacc as bacc
import concourse.mybir as mybir
import concourse.tile as tile
from concourse.bass_utils import run_bass_kernel_spmd

BF16 = ml_dtypes.bfloat16
FP8 = ml_dtypes.float8_e4m3      # TRN e4m3: max normal 240

N_CORES = 8
B, L, K = 64, 256, 1024          # batch, seq len (La=Lb), feature dim
BPC = B // N_CORES               # batches per core
G = BPC // 2                     # batch-pair groups per core
KC = K // 128                    # feature chunks of 128
N8 = 4                           # fp8 feature chunks (rest bf16): 4/8 split
C8 = N8 // 2                     # DoubleRow pair-instructions per m-chunk
NB16 = KC - N8                   # bf16 feature chunks
F32 = mybir.dt.float32
DBF = mybir.dt.bfloat16
F8 = mybir.dt.float8e4
Act = mybir.ActivationFunctionType
PM = mybir.MatmulPerfMode


def _build_program(sm_inv):
    # Bacc (not raw Bass): its compile() legalizes multi-wait instructions
    # (TRN2 allows at most one sync wait per instruction).
    nc = bacc.Bacc("TRN2", debug=False, target_bir_lowering=False)

    at = nc.dram_tensor("at", [G, 128, KC, 2 * L], DBF, kind="ExternalInput")
    bt8 = nc.dram_tensor("bt8", [G, 128, C8, 2, 2 * L], F8, kind="ExternalInput")
    bt16 = nc.dram_tensor("bt16", [G, 128, NB16, 2 * L], DBF, kind="ExternalInput")
    mt8 = nc.dram_tensor("mt8", [KC, 128, C8, 2, 128], F8, kind="ExternalInput")
    mt16 = nc.dram_tensor("mt16", [KC, 128, NB16, 128], DBF, kind="ExternalInput")
    u = nc.dram_tensor("u", [128, KC], F32, kind="ExternalInput")
    bu = nc.dram_tensor("bu", [1, BPC * L], DBF, kind="ExternalInput")
    probs = nc.dram_tensor("probs", [BPC, L, L], F32, kind="ExternalOutput")

    with tile.TileContext(nc) as tc:
        with (
            tc.tile_pool(name="consts", bufs=1) as consts,
            tc.tile_pool(name="inp", bufs=2) as in_pool,
            tc.tile_pool(name="tt", bufs=10) as tt_pool,
            tc.tile_pool(name="sm", bufs=4) as sm_pool,
            tc.tile_pool(name="small", bufs=4) as small,
            tc.tile_pool(name="ps_tt", bufs=6, space="PSUM") as ps_tt,
            tc.tile_pool(name="ps_sc", bufs=2, space="PSUM") as ps_sc,
        ):
            u_sb = consts.tile([128, KC], F32)
            nc.sync.dma_start(out=u_sb, in_=u[:, :])
            bu_sb = consts.tile([1, BPC * L], DBF)
            nc.sync.dma_start(out=bu_sb, in_=bu[:, :])
            mt8_sb = consts.tile([128, KC, C8, 2, 128], F8)
            mt16_sb = consts.tile([128, KC, NB16, 128], DBF)
            ones_row_bf = consts.tile([1, 128], DBF)
            nc.vector.memset(ones_row_bf, 1.0)

            from concourse import library_config
            nc.gpsimd.load_library(library_config.attnmlp)

            # PE warm-up: dummy matmuls while the first DMAs land, so the HAM
            # clock gate is already released when real matmuls start.
            warm_sb = consts.tile([128, 2 * L], DBF)
            nc.vector.memset(warm_sb, 0.0)
            warm_ps = ps_sc.tile([128, 2 * L], F32, tag="sc")
            for i in range(5):
                nc.tensor.matmul(
                    warm_ps, warm_sb[:, 0:128], warm_sb,
                    start=(i == 0), stop=(i == 4),
                )

            for g in range(G):
                bt8_sb = in_pool.tile([128, C8, 2, 2 * L], F8, tag="bt8")
                nc.sync.dma_start(out=bt8_sb, in_=bt8[g])
                if g == 0:
                    for m in range(KC):
                        nc.sync.dma_start(out=mt8_sb[:, m], in_=mt8[m])
                bt16_sb = in_pool.tile([128, NB16, 2 * L], DBF, tag="bt16")
                nc.sync.dma_start(out=bt16_sb, in_=bt16[g])
                if g == 0:
                    for m in range(KC):
                        nc.sync.dma_start(out=mt16_sb[:, m], in_=mt16[m])
                at_sb = in_pool.tile([128, KC, 2 * L], DBF, tag="at")
                nc.sync.dma_start(out=at_sb, in_=at[g])

                # Phase 1: all 8 TT chunks (kept in SBUF; tt_pool holds them)
                tt_chunks = []
                for m in range(KC):
                    tt_ps = ps_tt.tile([128, 2 * L], F32, tag="tt_ps")
                    for c in range(C8):
                        nc.tensor.matmul(
                            tt_ps, mt8_sb[:, m, c], bt8_sb[:, c],
                            start=(c == 0), stop=False,
                            perf_mode=PM.DoubleRow,
                        )
                    for l in range(NB16):
                        nc.tensor.matmul(
                            tt_ps, mt16_sb[:, m, l], bt16_sb[:, l],
                            start=False, stop=(l == NB16 - 1),
                        )
                    tt_sb = tt_pool.tile([128, 2 * L], DBF, tag="tt")
                    # TT' = TT + sM*u[chunk m] (folds the a.u rank-1 term); DVE
                    # (not ACT) so the scalar engine never swaps LUT tables.
                    nc.vector.tensor_scalar_add(tt_sb, tt_ps, u_sb[:, m : m + 1])
                    tt_chunks.append(tt_sb)

                # Phase 2: scores per batch in ONE psum bank (sequential h
                # accumulation groups), then a single fused exp+rowsum.
                for q in range(2):
                    bq = 2 * g + q
                    sc_ps = ps_sc.tile([128, 2 * L], F32, tag="sc")
                    for h in range(2):
                        for m in range(KC):
                            nc.tensor.matmul(
                                sc_ps[:, h * L : (h + 1) * L],
                                at_sb[:, m, q * L + h * 128 : q * L + h * 128 + 128],
                                tt_chunks[m][:, q * L : (q + 1) * L],
                                start=(m == 0), stop=False,
                            )
                        # inject bu (K=1 accumulate): S[i, j] += 1 * sM*bu[j]
                        nc.tensor.matmul(
                            sc_ps[:, h * L : (h + 1) * L],
                            ones_row_bf, bu_sb[:, bq * L : (bq + 1) * L],
                            start=False, stop=True,
                        )

                    # ---- softmax over the whole (256, 256) grid per batch ----
                    exp_sb = sm_pool.tile([128, 2 * L], F32, tag="exp")
                    colsum = small.tile([128, 1], F32, tag="cs")
                    nc.scalar.activation(
                        exp_sb, sc_ps, Act.Exp, scale=float(sm_inv),
                        accum_out=colsum,
                    )
                    # total over partitions, broadcast to all (GpSimd), recip
                    tot_col = small.tile([128, 1], F32, tag="totc")
                    nc.gpsimd.partition_all_reduce(
                        tot_col, colsum, channels=128,
                        reduce_op=bass.bass_isa.ReduceOp.add,
                    )
                    rcp_col = small.tile([128, 1], F32, tag="rcpc")
                    nc.vector.reciprocal(rcp_col, tot_col)
                    probs_sb = sm_pool.tile([128, 2, L], F32, tag="probs")
                    for h in range(2):
                        # split by half so the first DMA overlaps the second mul
                        nc.vector.tensor_scalar_mul(
                            probs_sb[:, h],
                            exp_sb[:, h * L : (h + 1) * L],
                            rcp_col,
                        )
                        nc.sync.dma_start(
                            out=probs[bq][h * 128 : (h + 1) * 128, :],
                            in_=probs_sb[:, h],
                        )
    return nc


def _prep_host(a, b, Wa, ba, Wb, bb, w, wbias):
    """Weight folding (f64) + per-core shards: mixed fp8/bf16 feature-major."""
    Wa64 = Wa.astype(np.float64)
    Wb64 = Wb.astype(np.float64)
    w64 = w.astype(np.float64)
    M = (Wa64 * w64[None, :]) @ Wb64.T                  # (K, K)
    u64 = (Wa64 * w64[None, :]) @ bb.astype(np.float64)
    v64 = (Wb64 * w64[None, :]) @ ba.astype(np.float64)

    sM = 2.0 ** np.floor(np.log2(239.0 / np.abs(M).max()))
    Ms = M * sM                                          # scaled fold

    # mt8[m, p, c, i, km] = sM*M[m*128+km, (2c+i)*128+p]  (fp8 chunks 0..N8-1)
    # mt16[m, p, l, km]   = sM*M[m*128+km, (N8+l)*128+p]  (bf16 chunks)
    Mb = Ms.reshape(KC, 128, KC, 128)                    # [m, km, lc, p]
    mt8_np = np.ascontiguousarray(
        Mb[:, :, :N8, :].reshape(KC, 128, C8, 2, 128).transpose(0, 4, 2, 3, 1)
    ).astype(FP8)
    mt16_np = np.ascontiguousarray(
        Mb[:, :, N8:, :].transpose(0, 3, 2, 1)
    ).astype(BF16)

    u_np = np.ascontiguousarray(
        (u64 * sM).astype(np.float32).reshape(KC, 128).T
    )                                                    # [p, c]

    # bu[b, j] = v . b[b, j, :], host rank-1 fold (scaled)
    bu_all = (b.astype(np.float64) @ v64) * sM           # (B, L)

    in_maps = []
    for cidx in range(N_CORES):
        sl = slice(cidx * BPC, (cidx + 1) * BPC)
        a_c, b_c = a[sl], b[sl]
        # feature-major, batch pairs side by side: x_fm[g, k, q*L+j]
        def fm(x):
            xt = x.transpose(0, 2, 1)                    # (BPC, K, L)
            return xt.reshape(G, 2, K, L).transpose(0, 2, 1, 3).reshape(G, K, 2 * L)
        a_fm = fm(a_c)
        b_fm = fm(b_c)
        at_np = np.ascontiguousarray(
            a_fm.reshape(G, KC, 128, 2 * L).transpose(0, 2, 1, 3)
        ).astype(BF16)
        b8 = b_fm[:, : N8 * 128, :].reshape(G, C8, 2, 128, 2 * L)
        bt8_np = np.ascontiguousarray(b8.transpose(0, 3, 1, 2, 4)).astype(FP8)
        b16 = b_fm[:, N8 * 128 :, :].reshape(G, NB16, 128, 2 * L)
        bt16_np = np.ascontiguousarray(b16.transpose(0, 2, 1, 3)).astype(BF16)
        bu_np = np.ascontiguousarray(
            bu_all[sl].reshape(1, BPC * L)
        ).astype(BF16)
        in_maps.append(
            {
                "at": at_np,
                "bt8": bt8_np,
                "bt16": bt16_np,
                "mt8": mt8_np,
                "mt16": mt16_np,
                "u": u_np,
                "bu": bu_np,
            }
        )
    return in_maps, 1.0 / sM


def _run(inputs, trace=False):
    in_maps, sm_inv = _prep_host(**inputs)
    nc = _build_program(sm_inv)
    nc.compile()
    res = run_bass_kernel_spmd(
        nc, in_maps, core_ids=list(range(N_CORES)), trace=trace
    )
    out = np.concatenate([res.results[c]["probs"] for c in range(N_CORES)], axis=0)
    return out.astype(np.float32), res


def kernel(**inputs) -> np.ndarray:
    out, _ = _run(inputs, trace=False)
    return out


# revision 3
# speedup vs baseline: 1.2143x; 1.0272x over previous
"""Bass/Trainium2 kernel for fused bilinear attention + softmax.

reference computation:
    pa = a @ Wa + ba                      (B, La, D)
    pb = b @ Wb + bb                      (B, Lb, D)
    scores = einsum('bid,bjd->bij', pa * w, pb) + wbias
    out = softmax(scores.reshape(B, La*Lb)).reshape(B, La, Lb)

Device strategy (8 NeuronCores, data-parallel over batch, 8 batches/core):
    Weight-only host folding:  M = (Wa*w) @ Wb.T,  u = (Wa*w)@bb,  v = (Wb*w)@ba
      scores[b,i,j] = a_i M b_j^T + (a_i.u) + (b_j.v) + const
    const (+wbias) dropped: softmax over the flattened grid is shift-invariant.
    bu[b,j] = v . b_j is a rank-1 term computed on host (like u/v folding).

    Everything device-side carries a power-of-2 scale sM on M (so the fp8
    chunks use the e4m3 range); exp() unscales via its scale operand.

    Per pair of batches (rhs free dim 512):
      TT   = (sM*M) @ bT + sM*u   mixed-precision contraction:
             first N8 feature chunks as fp8e4m3 DoubleRow pair-matmuls
             (2 chunks per instruction, 2x PE throughput), the rest bf16.
             DVE eviction to bf16 adds sM*u.
      S    = aT^T @ TT + 1(x)(sM*bu)  bf16 matmuls (N=256) + K=1 inject
      softmax: ACT exp(S/sM) with accum_out rowsum -> f32 ones-matmul on PE
               (sums+broadcasts over partitions) -> DVE reciprocal ->
               DVE scale -> one DMA per batch
    Group 0 runs all its fp8 DoubleRow matmuls before the bf16 ones so the
    PE can start as soon as the small fp8 DMAs land.
    PE warm-up matmuls run during the initial DMAs (HAM clock-gate release).
"""

import numpy as np
import ml_dtypes

import concourse.bass as bass
import concourse.bacc as bacc
import concourse.mybir as mybir
import concourse.tile as tile
from concourse.bass_utils import run_bass_kernel_spmd

BF16 = ml_dtypes.bfloat16
FP8 = ml_dtypes.float8_e4m3      # TRN e4m3: max normal 240

N_CORES = 8
B, L, K = 64, 256, 1024          # batch, seq len (La=Lb), feature dim
BPC = B // N_CORES               # batches per core
G = BPC // 2                     # batch-pair groups per core
KC = K // 128                    # feature chunks of 128
N8 = 4                           # fp8 feature chunks (rest bf16): 4/8 split
C8 = N8 // 2                     # DoubleRow pair-instructions per m-chunk
NB16 = KC - N8                   # bf16 feature chunks
F32 = mybir.dt.float32
DBF = mybir.dt.bfloat16
F8 = mybir.dt.float8e4
Act = mybir.ActivationFunctionType
PM = mybir.MatmulPerfMode


def _build_program(sm_inv):
    # Bacc (not raw Bass): its compile() legalizes multi-wait instructions
    # (TRN2 allows at most one sync wait per instruction).
    nc = bacc.Bacc("TRN2", debug=False, target_bir_lowering=False)

    at = nc.dram_tensor("at", [G, 128, KC, 2 * L], DBF, kind="ExternalInput")
    bt8 = nc.dram_tensor("bt8", [G, 128, C8, 2, 2 * L], F8, kind="ExternalInput")
    bt16 = nc.dram_tensor("bt16", [G, 128, NB16, 2 * L], DBF, kind="ExternalInput")
    # fp8 M blocks in 2 descriptors, bf16 M blocks in 4
    mt8 = nc.dram_tensor("mt8", [2, 128, KC // 2, C8, 2, 128], F8, kind="ExternalInput")
    mt16 = nc.dram_tensor("mt16", [4, 128, KC // 4, NB16, 128], DBF, kind="ExternalInput")
    u = nc.dram_tensor("u", [128, KC], F32, kind="ExternalInput")
    bu = nc.dram_tensor("bu", [1, BPC * L], DBF, kind="ExternalInput")
    probs = nc.dram_tensor("probs", [BPC, L, L], F32, kind="ExternalOutput")

    with tile.TileContext(nc) as tc:
        with (
            tc.tile_pool(name="consts", bufs=1) as consts,
            tc.tile_pool(name="inp", bufs=2) as in_pool,
            tc.tile_pool(name="tt", bufs=10) as tt_pool,
            tc.tile_pool(name="sm", bufs=4) as sm_pool,
            tc.tile_pool(name="small", bufs=4) as small,
            tc.tile_pool(name="ps_tt", bufs=6, space="PSUM") as ps_tt,
            tc.tile_pool(name="ps_sc", bufs=2, space="PSUM") as ps_sc,
        ):
            mt8_sb = consts.tile([128, KC, C8, 2, 128], F8)
            mt16_sb = consts.tile([128, KC, NB16, 128], DBF)
            u_sb = consts.tile([128, KC], F32)
            bu_sb = consts.tile([1, BPC * L], DBF)
            ones_row_bf = consts.tile([1, 128], DBF)
            nc.vector.memset(ones_row_bf, 1.0)
            ones_sq_f32 = consts.tile([128, 128], F32)
            nc.vector.memset(ones_sq_f32, 1.0)

            # PE warm-up: dummy matmuls while the first DMAs land, so the HAM
            # clock gate is already released when real matmuls start.
            warm_sb = consts.tile([128, 2 * L], DBF)
            nc.vector.memset(warm_sb, 0.0)
            warm_ps = ps_sc.tile([128, 2 * L], F32, tag="sc")
            for i in range(3):
                nc.tensor.matmul(
                    warm_ps, warm_sb[:, 0:128], warm_sb,
                    start=(i == 0), stop=(i == 2),
                )

            for g in range(G):
                bt8_sb = in_pool.tile([128, C8, 2, 2 * L], F8, tag="bt8")
                nc.sync.dma_start(out=bt8_sb, in_=bt8[g])
                if g == 0:
                    for j in range(2):
                        nc.sync.dma_start(
                            out=mt8_sb[:, j * (KC // 2) : (j + 1) * (KC // 2)],
                            in_=mt8[j],
                        )
                bt16_sb = in_pool.tile([128, NB16, 2 * L], DBF, tag="bt16")
                nc.sync.dma_start(out=bt16_sb, in_=bt16[g])
                if g == 0:
                    for j in range(4):
                        nc.sync.dma_start(
                            out=mt16_sb[:, j * (KC // 4) : (j + 1) * (KC // 4)],
                            in_=mt16[j],
                        )
                    nc.sync.dma_start(out=u_sb, in_=u[:, :])
                    nc.sync.dma_start(out=bu_sb, in_=bu[:, :])
                at_sb = in_pool.tile([128, KC, 2 * L], DBF, tag="at")
                nc.sync.dma_start(out=at_sb, in_=at[g])

                # Phase 1: all 8 TT chunks (kept in SBUF; tt_pool holds them).
                # Group 0: fp8 DoubleRow sweep first across every m (it only
                # needs the small fp8 DMAs), bf16 sweep after; m=6,7 borrow
                # the ps_sc banks, which are idle until phase 2.
                tt_ps_tiles = []
                for m in range(KC):
                    pool = ps_tt if m < 6 or g > 0 else ps_sc
                    tag = "tt_ps" if m < 6 or g > 0 else "sc"
                    tt_ps = pool.tile([128, 2 * L], F32, tag=tag)
                    tt_ps_tiles.append(tt_ps)
                    for c in range(C8):
                        nc.tensor.matmul(
                            tt_ps, mt8_sb[:, m, c], bt8_sb[:, c],
                            start=(c == 0), stop=False,
                            perf_mode=PM.DoubleRow,
                            skip_group_check=True,
                        )
                    if g > 0:
                        tt_ps_tiles[m] = None
                        for l in range(NB16):
                            nc.tensor.matmul(
                                tt_ps, mt16_sb[:, m, l], bt16_sb[:, l],
                                start=False, stop=(l == NB16 - 1),
                                skip_group_check=True,
                            )
                        tt_sb = tt_pool.tile([128, 2 * L], DBF, tag="tt")
                        # TT' = TT + sM*u[m] (folds a.u); DVE so the scalar
                        # engine never swaps LUT tables.
                        nc.vector.tensor_scalar_add(
                            tt_sb, tt_ps, u_sb[:, m : m + 1]
                        )
                        if m == 0:
                            tt_chunks = []
                        tt_chunks.append(tt_sb)
                if g == 0:
                    tt_chunks = []
                    for m in range(KC):
                        tt_ps = tt_ps_tiles[m]
                        for l in range(NB16):
                            nc.tensor.matmul(
                                tt_ps, mt16_sb[:, m, l], bt16_sb[:, l],
                                start=False, stop=(l == NB16 - 1),
                                skip_group_check=True,
                            )
                        tt_sb = tt_pool.tile([128, 2 * L], DBF, tag="tt")
                        nc.vector.tensor_scalar_add(
                            tt_sb, tt_ps, u_sb[:, m : m + 1]
                        )
                        tt_chunks.append(tt_sb)

                # Phase 2: scores per batch in ONE psum bank (sequential h
                # accumulation groups), then a single fused exp+rowsum.
                for q in range(2):
                    bq = 2 * g + q
                    sc_ps = ps_sc.tile([128, 2 * L], F32, tag="sc")
                    for h in range(2):
                        for m in range(KC):
                            nc.tensor.matmul(
                                sc_ps[:, h * L : (h + 1) * L],
                                at_sb[:, m, q * L + h * 128 : q * L + h * 128 + 128],
                                tt_chunks[m][:, q * L : (q + 1) * L],
                                start=(m == 0), stop=False,
                            )
                        # inject bu (K=1 accumulate): S[i, j] += 1 * sM*bu[j]
                        nc.tensor.matmul(
                            sc_ps[:, h * L : (h + 1) * L],
                            ones_row_bf, bu_sb[:, bq * L : (bq + 1) * L],
                            start=False, stop=True,
                        )

                    # ---- softmax over the whole (256, 256) grid per batch ----
                    exp_sb = sm_pool.tile([128, 2 * L], F32, tag="exp")
                    colsum = small.tile([128, 1], F32, tag="cs")
                    nc.scalar.activation(
                        exp_sb, sc_ps, Act.Exp, scale=float(sm_inv),
                        accum_out=colsum,
                    )
                    # total over partitions: f32 ones-matmul sums colsum and
                    # broadcasts to every partition (PE is idle here; avoids
                    # a GpSimd round-trip). Reuses sc_ps after exp consumed it.
                    nc.tensor.matmul(
                        sc_ps[:, 0:1], ones_sq_f32, colsum,
                        start=True, stop=True, skip_group_check=True,
                    )
                    rcp_col = small.tile([128, 1], F32, tag="rcpc")
                    nc.vector.reciprocal(rcp_col, sc_ps[:, 0:1])
                    probs_sb = sm_pool.tile([128, 2, L], F32, tag="probs")
                    for h in range(2):
                        nc.vector.tensor_scalar_mul(
                            probs_sb[:, h],
                            exp_sb[:, h * L : (h + 1) * L],
                            rcp_col,
                        )
                    nc.sync.dma_start(
                        out=probs[bq].rearrange("(h p) n -> p h n", p=128),
                        in_=probs_sb,
                    )
    return nc


def _prep_host(a, b, Wa, ba, Wb, bb, w, wbias):
    """Weight folding (f64) + per-core shards: mixed fp8/bf16 feature-major."""
    Wa64 = Wa.astype(np.float64)
    Wb64 = Wb.astype(np.float64)
    w64 = w.astype(np.float64)
    M = (Wa64 * w64[None, :]) @ Wb64.T                  # (K, K)
    u64 = (Wa64 * w64[None, :]) @ bb.astype(np.float64)
    v64 = (Wb64 * w64[None, :]) @ ba.astype(np.float64)

    sM = 2.0 ** np.floor(np.log2(239.0 / np.abs(M).max()))
    Ms = M * sM                                          # scaled fold

    # mt8[j, p, m', c, i, km] = sM*M[(4j+m')*128+km, (2c+i)*128+p]
    # mt16[j, p, m', l, km]   = sM*M[(2j+m')*128+km, (N8+l)*128+p]
    Mb = Ms.reshape(KC, 128, KC, 128)                    # [m, km, lc, p]
    mt8_np = np.ascontiguousarray(
        Mb[:, :, :N8, :]
        .reshape(2, KC // 2, 128, C8, 2, 128)
        .transpose(0, 5, 1, 3, 4, 2)
    ).astype(FP8)
    mt16_np = np.ascontiguousarray(
        Mb[:, :, N8:, :]
        .reshape(4, KC // 4, 128, NB16, 128)
        .transpose(0, 4, 1, 3, 2)
    ).astype(BF16)

    u_np = np.ascontiguousarray(
        (u64 * sM).astype(np.float32).reshape(KC, 128).T
    )                                                    # [p, c]

    # bu[b, j] = v . b[b, j, :], host rank-1 fold (scaled)
    bu_all = (b.astype(np.float64) @ v64) * sM           # (B, L)

    in_maps = []
    for cidx in range(N_CORES):
        sl = slice(cidx * BPC, (cidx + 1) * BPC)
        a_c, b_c = a[sl], b[sl]
        # feature-major, batch pairs side by side: x_fm[g, k, q*L+j]
        def fm(x):
            xt = x.transpose(0, 2, 1)                    # (BPC, K, L)
            return xt.reshape(G, 2, K, L).transpose(0, 2, 1, 3).reshape(G, K, 2 * L)
        a_fm = fm(a_c)
        b_fm = fm(b_c)
        at_np = np.ascontiguousarray(
            a_fm.reshape(G, KC, 128, 2 * L).transpose(0, 2, 1, 3)
        ).astype(BF16)
        b8 = b_fm[:, : N8 * 128, :].reshape(G, C8, 2, 128, 2 * L)
        bt8_np = np.ascontiguousarray(b8.transpose(0, 3, 1, 2, 4)).astype(FP8)
        b16 = b_fm[:, N8 * 128 :, :].reshape(G, NB16, 128, 2 * L)
        bt16_np = np.ascontiguousarray(b16.transpose(0, 2, 1, 3)).astype(BF16)
        bu_np = np.ascontiguousarray(
            bu_all[sl].reshape(1, BPC * L)
        ).astype(BF16)
        in_maps.append(
            {
                "at": at_np,
                "bt8": bt8_np,
                "bt16": bt16_np,
                "mt8": mt8_np,
                "mt16": mt16_np,
                "u": u_np,
                "bu": bu_np,
            }
        )
    return in_maps, 1.0 / sM


def _run(inputs, trace=False):
    in_maps, sm_inv = _prep_host(**inputs)
    nc = _build_program(sm_inv)
    nc.compile()
    res = run_bass_kernel_spmd(
        nc, in_maps, core_ids=list(range(N_CORES)), trace=trace
    )
    out = np.concatenate([res.results[c]["probs"] for c in range(N_CORES)], axis=0)
    return out.astype(np.float32), res


def kernel(**inputs) -> np.ndarray:
    out, _ = _run(inputs, trace=False)
    return out


# revision 4
# speedup vs baseline: 1.2638x; 1.0408x over previous
"""Bass/Trainium2 kernel for fused bilinear attention + softmax.

reference computation:
    pa = a @ Wa + ba                      (B, La, D)
    pb = b @ Wb + bb                      (B, Lb, D)
    scores = einsum('bid,bjd->bij', pa * w, pb) + wbias
    out = softmax(scores.reshape(B, La*Lb)).reshape(B, La, Lb)

Device strategy (8 NeuronCores, data-parallel over batch, 8 batches/core):
    Weight-only host folding:  M = (Wa*w) @ Wb.T,  u = (Wa*w)@bb,  v = (Wb*w)@ba
      scores[b,i,j] = a_i M b_j^T + (a_i.u) + (b_j.v) + const
    const (+wbias) dropped: softmax over the flattened grid is shift-invariant.
    bu[b,j] = v . b_j is a rank-1 term computed on host (like u/v folding).

    Everything device-side carries a power-of-2 scale sM on M (so the fp8
    chunks use the e4m3 range); exp() unscales via its scale operand.

    Per pair of batches (rhs free dim 512):
      TT   = (sM*M) @ bT + sM*u   mixed-precision contraction:
             first N8 feature chunks as fp8e4m3 DoubleRow pair-matmuls
             (2 chunks per instruction, 2x PE throughput), the rest bf16.
             DVE eviction to bf16 adds sM*u.
      S    = aT^T @ TT + 1(x)(sM*bu)  bf16 matmuls (N=256) + K=1 inject
      softmax: per-half ACT exp(S/sM) with accum_out rowsum (first half
               overlaps the second half's score matmuls) -> f32 ones-matmul
               on PE accumulates+broadcasts the total -> DVE reciprocal ->
               DVE scale -> per-half DMA out
    Group 0 runs all its fp8 DoubleRow matmuls before the bf16 ones, and its
    input DMAs are split across the Sync and Scalar HWDGE queues, so the PE
    starts as soon as the small fp8 descriptors land.
    PE warm-up matmuls run during the initial DMAs (HAM clock-gate release).
"""

import numpy as np
import ml_dtypes

import concourse.bass as bass
import concourse.bacc as bacc
import concourse.mybir as mybir
import concourse.tile as tile
from concourse.bass_utils import run_bass_kernel_spmd

BF16 = ml_dtypes.bfloat16
FP8 = ml_dtypes.float8_e4m3      # TRN e4m3: max normal 240

N_CORES = 8
B, L, K = 64, 256, 1024          # batch, seq len (La=Lb), feature dim
BPC = B // N_CORES               # batches per core
G = BPC // 2                     # batch-pair groups per core
KC = K // 128                    # feature chunks of 128
N8 = 6                           # fp8 feature chunks (rest bf16): 6/8 split
C8 = N8 // 2                     # DoubleRow pair-instructions per m-chunk
NB16 = KC - N8                   # bf16 feature chunks
F32 = mybir.dt.float32
DBF = mybir.dt.bfloat16
F8 = mybir.dt.float8e4
Act = mybir.ActivationFunctionType
PM = mybir.MatmulPerfMode


def _build_program(sm_inv):
    # Bacc (not raw Bass): its compile() legalizes multi-wait instructions
    # (TRN2 allows at most one sync wait per instruction).
    nc = bacc.Bacc("TRN2", debug=False, target_bir_lowering=False)

    at = nc.dram_tensor("at", [G, 128, KC, 2 * L], DBF, kind="ExternalInput")
    bt8 = nc.dram_tensor("bt8", [G, 128, C8, 2, 2 * L], F8, kind="ExternalInput")
    bt16 = nc.dram_tensor("bt16", [G, 128, NB16, 2 * L], DBF, kind="ExternalInput")
    # fp8 M blocks in 2 descriptors, bf16 M blocks in 2
    mt8 = nc.dram_tensor("mt8", [2, 128, KC // 2, C8, 2, 128], F8, kind="ExternalInput")
    mt16 = nc.dram_tensor("mt16", [2, 128, KC // 2, NB16, 128], DBF, kind="ExternalInput")
    u = nc.dram_tensor("u", [128, KC], F32, kind="ExternalInput")
    bu = nc.dram_tensor("bu", [1, BPC * L], DBF, kind="ExternalInput")
    probs = nc.dram_tensor("probs", [BPC, L, L], F32, kind="ExternalOutput")

    with tile.TileContext(nc) as tc:
        with (
            tc.tile_pool(name="consts", bufs=1) as consts,
            tc.tile_pool(name="inp", bufs=2) as in_pool,
            tc.tile_pool(name="tt", bufs=10) as tt_pool,
            tc.tile_pool(name="sm", bufs=4) as sm_pool,
            tc.tile_pool(name="small", bufs=4) as small,
            tc.tile_pool(name="ps_tt", bufs=6, space="PSUM") as ps_tt,
            tc.tile_pool(name="ps_sc", bufs=2, space="PSUM") as ps_sc,
        ):
            mt8_sb = consts.tile([128, KC, C8, 2, 128], F8)
            mt16_sb = consts.tile([128, KC, NB16, 128], DBF)
            u_sb = consts.tile([128, KC], F32)
            bu_sb = consts.tile([1, BPC * L], DBF)
            ones_row_bf = consts.tile([1, 128], DBF)
            nc.vector.memset(ones_row_bf, 1.0)
            ones_sq_f32 = consts.tile([128, 128], F32)
            nc.vector.memset(ones_sq_f32, 1.0)

            # PE warm-up: dummy matmuls while the first DMAs land, so the HAM
            # clock gate is already released when real matmuls start.
            warm_sb = consts.tile([128, 128], DBF)
            nc.vector.memset(warm_sb, 0.0)
            warm_ps = ps_sc.tile([128, 2 * L], F32, tag="sc")
            for i in range(3):
                nc.tensor.matmul(
                    warm_ps[:, 0:128], warm_sb, warm_sb,
                    start=(i == 0), stop=(i == 2),
                )

            for g in range(G):
                bt8_sb = in_pool.tile([128, C8, 2, 2 * L], F8, tag="bt8")
                nc.sync.dma_start(out=bt8_sb, in_=bt8[g])
                if g == 0:
                    # group-0 fill split across both HWDGE queues: Sync carries
                    # what the fp8 sweep needs, Scalar the bf16 side + phase 2.
                    for j in range(2):
                        nc.sync.dma_start(
                            out=mt8_sb[:, j * (KC // 2) : (j + 1) * (KC // 2)],
                            in_=mt8[j],
                        )
                bt16_sb = in_pool.tile([128, NB16, 2 * L], DBF, tag="bt16")
                at_sb = in_pool.tile([128, KC, 2 * L], DBF, tag="at")
                if g == 0:
                    nc.scalar.dma_start(out=bt16_sb, in_=bt16[g])
                    for j in range(2):
                        nc.scalar.dma_start(
                            out=mt16_sb[:, j * (KC // 2) : (j + 1) * (KC // 2)],
                            in_=mt16[j],
                        )
                    nc.scalar.dma_start(out=u_sb, in_=u[:, :])
                    nc.scalar.dma_start(out=bu_sb, in_=bu[:, :])
                    nc.scalar.dma_start(out=at_sb, in_=at[g])
                else:
                    nc.sync.dma_start(out=bt16_sb, in_=bt16[g])
                    nc.sync.dma_start(out=at_sb, in_=at[g])

                # Phase 1: all 8 TT chunks (kept in SBUF; tt_pool holds them).
                # Group 0: fp8 DoubleRow sweep first across every m (it only
                # needs the Sync-queue DMAs), bf16 sweep after; m=6,7 borrow
                # the ps_sc banks, which are idle until phase 2.
                tt_ps_tiles = []
                tt_chunks = []
                for m in range(KC):
                    pool = ps_tt if m < 6 or g > 0 else ps_sc
                    tag = "tt_ps" if m < 6 or g > 0 else "sc"
                    tt_ps = pool.tile([128, 2 * L], F32, tag=tag)
                    tt_ps_tiles.append(tt_ps)
                    for c in range(C8):
                        nc.tensor.matmul(
                            tt_ps, mt8_sb[:, m, c], bt8_sb[:, c],
                            start=(c == 0), stop=False,
                            perf_mode=PM.DoubleRow,
                            skip_group_check=True,
                        )
                    if g > 0:
                        for l in range(NB16):
                            nc.tensor.matmul(
                                tt_ps, mt16_sb[:, m, l], bt16_sb[:, l],
                                start=False, stop=(l == NB16 - 1),
                                skip_group_check=True,
                            )
                        tt_sb = tt_pool.tile([128, 2 * L], DBF, tag="tt")
                        # TT' = TT + sM*u[m] (folds a.u); DVE so the scalar
                        # engine never swaps LUT tables.
                        nc.vector.tensor_scalar_add(
                            tt_sb, tt_ps, u_sb[:, m : m + 1]
                        )
                        tt_chunks.append(tt_sb)
                if g == 0:
                    for m in range(KC):
                        tt_ps = tt_ps_tiles[m]
                        for l in range(NB16):
                            nc.tensor.matmul(
                                tt_ps, mt16_sb[:, m, l], bt16_sb[:, l],
                                start=False, stop=(l == NB16 - 1),
                                skip_group_check=True,
                            )
                        tt_sb = tt_pool.tile([128, 2 * L], DBF, tag="tt")
                        nc.vector.tensor_scalar_add(
                            tt_sb, tt_ps, u_sb[:, m : m + 1]
                        )
                        tt_chunks.append(tt_sb)

                # Phase 2: scores per batch in ONE psum bank (sequential h
                # accumulation groups), softmax split by half so the first
                # half's exp overlaps the second half's matmuls.
                for q in range(2):
                    bq = 2 * g + q
                    sc_ps = ps_sc.tile([128, 2 * L], F32, tag="sc")
                    exp_sb = sm_pool.tile([128, 2, L], F32, tag="exp")
                    colsum = small.tile([128, 2], F32, tag="cs")
                    for h in range(2):
                        for m in range(KC):
                            nc.tensor.matmul(
                                sc_ps[:, h * L : (h + 1) * L],
                                at_sb[:, m, q * L + h * 128 : q * L + h * 128 + 128],
                                tt_chunks[m][:, q * L : (q + 1) * L],
                                start=(m == 0), stop=False,
                            )
                        # inject bu (K=1 accumulate): S[i, j] += 1 * sM*bu[j]
                        nc.tensor.matmul(
                            sc_ps[:, h * L : (h + 1) * L],
                            ones_row_bf, bu_sb[:, bq * L : (bq + 1) * L],
                            start=False, stop=True,
                        )
                        nc.scalar.activation(
                            exp_sb[:, h], sc_ps[:, h * L : (h + 1) * L],
                            Act.Exp, scale=float(sm_inv),
                            accum_out=colsum[:, h : h + 1],
                        )

                    # total over partitions: f32 ones-matmuls sum both halves'
                    # rowsums and broadcast to every partition (PE is idle
                    # here; avoids a GpSimd round-trip). Fresh ps_tt tile —
                    # its ring slot is long recycled by now.
                    tot_ps = ps_tt.tile([128, 2 * L], F32, tag="tt_ps")
                    for h in range(2):
                        nc.tensor.matmul(
                            tot_ps[:, 0:1], ones_sq_f32, colsum[:, h : h + 1],
                            start=(h == 0), stop=(h == 1),
                            skip_group_check=True,
                        )
                    rcp_col = small.tile([128, 1], F32, tag="rcpc")
                    nc.vector.reciprocal(rcp_col, tot_ps[:, 0:1])
                    probs_sb = sm_pool.tile([128, 2, L], F32, tag="probs")
                    for h in range(2):
                        # split by half so the first DMA overlaps the second mul
                        nc.vector.tensor_scalar_mul(
                            probs_sb[:, h], exp_sb[:, h], rcp_col
                        )
                        nc.sync.dma_start(
                            out=probs[bq][h * 128 : (h + 1) * 128, :],
                            in_=probs_sb[:, h],
                        )
    return nc


def _prep_host(a, b, Wa, ba, Wb, bb, w, wbias):
    """Weight folding (f64) + per-core shards: mixed fp8/bf16 feature-major."""
    Wa64 = Wa.astype(np.float64)
    Wb64 = Wb.astype(np.float64)
    w64 = w.astype(np.float64)
    M = (Wa64 * w64[None, :]) @ Wb64.T                  # (K, K)
    u64 = (Wa64 * w64[None, :]) @ bb.astype(np.float64)
    v64 = (Wb64 * w64[None, :]) @ ba.astype(np.float64)

    sM = 2.0 ** np.floor(np.log2(239.0 / np.abs(M).max()))
    Ms = M * sM                                          # scaled fold

    # mt8[j, p, m', c, i, km] = sM*M[(4j+m')*128+km, (2c+i)*128+p]
    # mt16[j, p, m', l, km]   = sM*M[(4j+m')*128+km, (N8+l)*128+p]
    Mb = Ms.reshape(KC, 128, KC, 128)                    # [m, km, lc, p]
    mt8_np = np.ascontiguousarray(
        Mb[:, :, :N8, :]
        .reshape(2, KC // 2, 128, C8, 2, 128)
        .transpose(0, 5, 1, 3, 4, 2)
    ).astype(FP8)
    mt16_np = np.ascontiguousarray(
        Mb[:, :, N8:, :]
        .reshape(2, KC // 2, 128, NB16, 128)
        .transpose(0, 4, 1, 3, 2)
    ).astype(BF16)

    u_np = np.ascontiguousarray(
        (u64 * sM).astype(np.float32).reshape(KC, 128).T
    )                                                    # [p, c]

    # bu[b, j] = v . b[b, j, :], host rank-1 fold (scaled)
    bu_all = (b.astype(np.float64) @ v64) * sM           # (B, L)

    in_maps = []
    for cidx in range(N_CORES):
        sl = slice(cidx * BPC, (cidx + 1) * BPC)
        a_c, b_c = a[sl], b[sl]
        # feature-major, batch pairs side by side: x_fm[g, k, q*L+j]
        def fm(x):
            xt = x.transpose(0, 2, 1)                    # (BPC, K, L)
            return xt.reshape(G, 2, K, L).transpose(0, 2, 1, 3).reshape(G, K, 2 * L)
        a_fm = fm(a_c)
        b_fm = fm(b_c)
        at_np = np.ascontiguousarray(
            a_fm.reshape(G, KC, 128, 2 * L).transpose(0, 2, 1, 3)
        ).astype(BF16)
        b8 = b_fm[:, : N8 * 128, :].reshape(G, C8, 2, 128, 2 * L)
        bt8_np = np.ascontiguousarray(b8.transpose(0, 3, 1, 2, 4)).astype(FP8)
        b16 = b_fm[:, N8 * 128 :, :].reshape(G, NB16, 128, 2 * L)
        bt16_np = np.ascontiguousarray(b16.transpose(0, 2, 1, 3)).astype(BF16)
        bu_np = np.ascontiguousarray(
            bu_all[sl].reshape(1, BPC * L)
        ).astype(BF16)
        in_maps.append(
            {
                "at": at_np,
                "bt8": bt8_np,
                "bt16": bt16_np,
                "mt8": mt8_np,
                "mt16": mt16_np,
                "u": u_np,
                "bu": bu_np,
            }
        )
    return in_maps, 1.0 / sM


def _run(inputs, trace=False):
    in_maps, sm_inv = _prep_host(**inputs)
    nc = _build_program(sm_inv)
    nc.compile()
    res = run_bass_kernel_spmd(
        nc, in_maps, core_ids=list(range(N_CORES)), trace=trace
    )
    out = np.concatenate([res.results[c]["probs"] for c in range(N_CORES)], axis=0)
    return out.astype(np.float32), res


def kernel(**inputs) -> np.ndarray:
    out, _ = _run(inputs, trace=False)
    return out
